# revision 1
# baseline (speedup 1.0000x reference)
"""Trainium2 Bass kernel for nn_AttnBlock_Spatio_Temporal (B=4,T=5,C=512,H=W=32).

Distribution: 8 cores = (video b in 0..3) x (pixel-half h in 0..1).
Host rolls the HW axis per core so its own 512 pixels come first (spatial
attention / GroupNorm are permutation-invariant over key pixels). Each core
computes full-frame k/v but only its own queries; the temporal GroupNorm
needs full-frame stats -> tiny per-frame pair AllReduce of per-channel
(sum, sumsq). All heavy matmuls run in bf16 (fp32 accumulate); residual adds
stay fp32.

Channel-major layout: channel c lives at (partition p, block j) with
c = 4p + j, so DRAM rows stream in 16KB-contiguous runs (4x fewer DMA
descriptors) and GroupNorm groups (16 consecutive channels) = 4 consecutive
partitions. Weight matrices get their columns host-permuted to keep conv
outputs in the same convention with contiguous lhsT slices.
"""
import numpy as np

B, T, C, HW = 4, 5, 512, 1024
G = 32
EPS = 1e-6
P = 128
CB = C // P          # 4 channel blocks
HALF = HW // 2       # 512 own pixels
KB = HW // P         # 8 key-pixel blocks
QB = HALF // P       # 4 query/pixel blocks
SCALE = float(C) ** -0.5
INV_CNT = 1.0 / 16384.0   # per-group element count (16ch*1024px or 16ch*512px*2)

_CACHE = {}


def _build():
    import concourse.bacc as bacc
    import concourse.tile as tile
    import concourse.mybir as mybir

    f32 = mybir.dt.float32
    bf16 = mybir.dt.bfloat16
    MULT = mybir.AluOpType.mult
    ADD = mybir.AluOpType.add
    SUB = mybir.AluOpType.subtract
    AF = mybir.ActivationFunctionType
    AX = mybir.AxisListType

    nc = bacc.Bacc("TRN2", target_bir_lowering=False, debug=False, num_devices=8)

    x_d = nc.dram_tensor("x", [T, C, HW], f32, kind="ExternalInput").ap()
    w_names = ["wq", "wk", "wv", "wo", "wqt", "wkt", "wvt", "wot"]
    w_d = {nm: nc.dram_tensor(nm + "T", [C, C], bf16, kind="ExternalInput").ap()
           for nm in w_names}
    b_d = {nm: nc.dram_tensor(nm, [C], f32, kind="ExternalInput").ap()
           for nm in ["bq", "bk", "bv", "bo", "bot"]}
    bqt_d = nc.dram_tensor("bqt", [C], bf16, kind="ExternalInput").ap()
    g_d = {nm: nc.dram_tensor(nm, [C], f32, kind="ExternalInput").ap()
           for nm in ["gamma_s", "beta_s", "gamma_t", "beta_t"]}
    sel_d = nc.dram_tensor("sel4", [P, G], f32, kind="ExternalInput").ap()
    bc_d = nc.dram_tensor("bcast4", [G, P], f32, kind="ExternalInput").ap()
    out_d = nc.dram_tensor("out", [T, C, HALF], f32, kind="ExternalOutput").ap()

    def cpart(ap_1d):  # [C] dram -> [128, CB] tile order (c = p*CB + j)
        return ap_1d.rearrange("(p j) -> p j", p=P)

    with tile.TileContext(nc) as tc:
        with tc.tile_pool(name="consts", bufs=1) as consts, \
             tc.tile_pool(name="stat4", bufs=4) as stat4, \
             tc.tile_pool(name="spatio_p", bufs=T) as spatio_p, \
             tc.tile_pool(name="psum", bufs=6, space="PSUM") as psum, \
             tc.tile_pool(name="ps_aff", bufs=2, space="PSUM") as ps_aff, \
             tc.tile_pool(name="dram", bufs=T, space="DRAM") as dram:

            # ---------------- constants ----------------
            w_sb = {}
            for nm in w_names:
                w_sb[nm] = consts.tile([P, CB, C], bf16, tag="w_" + nm,
                                       name="w_" + nm)
            bias_sb = {}
            for nm in ["bq", "bk", "bv", "bo", "bot"]:
                bias_sb[nm] = consts.tile([P, CB], f32, tag="b_" + nm,
                                          name="b_" + nm)
                nc.sync.dma_start(out=bias_sb[nm], in_=cpart(b_d[nm]))
            gam_sb = {}
            for nm in ["gamma_s", "beta_s", "gamma_t", "beta_t"]:
                gam_sb[nm] = consts.tile([P, CB], f32, tag="g_" + nm,
                                         name="g_" + nm)
                nc.sync.dma_start(out=gam_sb[nm], in_=cpart(g_d[nm]))
            bqt_bc = consts.tile([P, C], bf16, tag="bqt_bc", name="bqt_bc")
            nc.sync.dma_start(out=bqt_bc, in_=bqt_d.unsqueeze(0).to_broadcast([P, C]))
            sel4 = consts.tile([P, G], f32, tag="sel4", name="sel4")
            nc.sync.dma_start(out=sel4, in_=sel_d)
            bcast4 = consts.tile([G, P], f32, tag="bcast4", name="bcast4")
            nc.sync.dma_start(out=bcast4, in_=bc_d)
            eps32 = consts.tile([G, 1], f32, tag="eps32", name="eps32")
            nc.vector.memset(eps32, EPS)

            def affine_core(g2, gamma, beta, scale_out, shift_out):
                """g2: [G, 2] group (sum, sumsq); writes scale/shift [P, CB]."""
                m2 = stat4.tile([G, 2], f32, tag="m2", name="m2")
                nc.scalar.activation(out=m2, in_=g2, func=AF.Copy, scale=INV_CNT)
                rg = stat4.tile([G, 2], f32, tag="rg", name="rg")
                nc.vector.tensor_tensor(out=rg[:, 0:1], in0=m2[:, 0:1],
                                        in1=m2[:, 0:1], op=MULT)
                nc.vector.tensor_tensor(out=rg[:, 0:1], in0=m2[:, 1:2],
                                        in1=rg[:, 0:1], op=SUB)
                nc.scalar.activation(out=rg[:, 0:1], in_=rg[:, 0:1],
                                     func=AF.Sqrt, bias=eps32, scale=1.0)
                nc.vector.reciprocal(rg[:, 0:1], rg[:, 0:1])
                # rg[:,1] = -mean * rstd
                nc.vector.tensor_scalar(out=rg[:, 1:2], in0=m2[:, 0:1],
                                        scalar1=rg[:, 0:1], scalar2=-1.0,
                                        op0=MULT, op1=MULT)
                ps_bc = ps_aff.tile([P, 512], f32, tag="ps_stat", name="ps_bc")
                nc.tensor.matmul(ps_bc[:, 0:2], bcast4[:, :], rg[:, :],
                                 start=True, stop=True)
                nc.vector.tensor_scalar_mul(out=scale_out, in0=gamma,
                                            scalar1=ps_bc[:, 0:1])
                nc.vector.scalar_tensor_tensor(out=shift_out, in0=gamma,
                                               scalar=ps_bc[:, 1:2], in1=beta,
                                               op0=MULT, op1=ADD)

            gnt = [None] * T
            spatio_tiles = [None] * T
            bounce_outs = [None] * T

            # ================= spatial phase =================
            with tc.tile_pool(name="spat2", bufs=2) as spat2, \
                 tc.tile_pool(name="spat1", bufs=1) as spat1:

                def temporal_tail(fi):
                    gsum = stat4.tile([G, 2], f32, tag="gsum", name="gsum")
                    nc.sync.dma_start(out=gsum[:], in_=bounce_outs[fi][:])
                    scale_t = stat4.tile([P, CB], f32, tag="scale_t",
                                         name="scale_t")
                    shift_t = stat4.tile([P, CB], f32, tag="shift_t",
                                         name="shift_t")
                    affine_core(gsum, gam_sb["gamma_t"], gam_sb["beta_t"],
                                scale_t, shift_t)
                    gnt[fi] = spatio_p.tile([P, CB, HALF], bf16, tag="gnt",
                                            name="gnt")
                    for j in range(CB):
                        nc.vector.tensor_scalar(
                            out=gnt[fi][:, j, :], in0=spatio_tiles[fi][:, j, :],
                            scalar1=scale_t[:, j:j + 1],
                            scalar2=shift_t[:, j:j + 1],
                            op0=MULT, op1=ADD)

                xfs = [None] * T
                hns = [None] * T

                def load_x(fi):
                    xf = spat2.tile([P, CB, HW], f32, tag="xf", name="xf")
                    nc.sync.dma_start(
                        out=xf, in_=x_d[fi].rearrange("(p j) hw -> p j hw", p=P))
                    xfs[fi] = xf

                def gn_block(fi):
                    if xfs[fi] is None:
                        load_x(fi)
                    xf = xfs[fi]
                    sums = spat2.tile([P, CB, 2], f32, tag="sums", name="sums")
                    sqj = spat2.tile([P, HW], f32, tag="sqj", name="sqj")
                    for j in range(CB):
                        nc.vector.tensor_reduce(out=sums[:, j, 0:1], in_=xf[:, j, :],
                                                axis=AX.X, op=ADD)
                        nc.scalar.activation(out=sqj, in_=xf[:, j, :],
                                             func=AF.Square,
                                             accum_out=sums[:, j, 1:2])
                    ps_g = ps_aff.tile([P, 512], f32, tag="ps_stat", name="ps_g")
                    nc.tensor.matmul(ps_g[0:G, 0:2 * CB], sel4[:, :],
                                     sums.rearrange("p j s -> p (j s)"),
                                     start=True, stop=True)
                    g2s = stat4.tile([G, 2], f32, tag="g2s", name="g2s")
                    nc.vector.tensor_reduce(
                        out=g2s,
                        in_=ps_g[0:G, 0:2 * CB].rearrange("g (j s) -> g s j", s=2),
                        axis=AX.X, op=ADD)
                    scale_s = stat4.tile([P, CB], f32, tag="scale_s", name="scale_s")
                    shift_s = stat4.tile([P, CB], f32, tag="shift_s", name="shift_s")
                    affine_core(g2s, gam_sb["gamma_s"], gam_sb["beta_s"],
                                scale_s, shift_s)
                    hn = spat2.tile([P, CB, HW], bf16, tag="hn", name="hn")
                    for j in range(CB):
                        nc.vector.tensor_scalar(
                            out=hn[:, j, :], in0=xf[:, j, :],
                            scalar1=scale_s[:, j:j + 1], scalar2=shift_s[:, j:j + 1],
                            op0=MULT, op1=ADD)
                    hns[fi] = hn

                gn_block(0)
                for nm in ["wk", "wq", "wv", "wo", "wqt", "wkt", "wvt", "wot"]:
                    nc.sync.dma_start(
                        out=w_sb[nm],
                        in_=w_d[nm].rearrange("(p kc) co -> p kc co", p=P))
                ks = [None] * T
                qs = [None] * T
                vs = [None] * T

                def conv_block(fi):
                    hnl = hns[fi]
                    k_sb = spat1.tile([P, CB, HW], bf16, tag="k_sb", name="k_sb",
                                      bufs=2)
                    for jo in range(CB):
                        for half in range(2):
                            ps = psum.tile([P, 512], f32, tag="psc", name="psc")
                            for kc in range(CB):
                                nc.tensor.matmul(
                                    ps[:, :], w_sb["wk"][:, kc, jo * P:(jo + 1) * P],
                                    hnl[:, kc, half * 512:(half + 1) * 512],
                                    start=(kc == 0), stop=(kc == CB - 1))
                            nc.vector.tensor_scalar_add(
                                out=k_sb[:, jo, half * 512:(half + 1) * 512],
                                in0=ps, scalar1=bias_sb["bk"][:, jo:jo + 1])
                    q_sb = spat1.tile([P, CB, HALF], bf16, tag="q_sb", name="q_sb",
                                      bufs=2)
                    for jo in range(CB):
                        ps = psum.tile([P, 512], f32, tag="psc", name="psc")
                        for kc in range(CB):
                            nc.tensor.matmul(
                                ps[:, :], w_sb["wq"][:, kc, jo * P:(jo + 1) * P],
                                hnl[:, kc, 0:HALF],
                                start=(kc == 0), stop=(kc == CB - 1))
                        nc.vector.tensor_scalar_add(
                            out=q_sb[:, jo, :], in0=ps,
                            scalar1=bias_sb["bq"][:, jo:jo + 1])
                    vT_sb = spat1.tile([P, KB, C], bf16, tag="vT_sb", name="vT_sb",
                                       bufs=2)
                    for pb in range(KB):
                        ps = psum.tile([P, 512], f32, tag="psc", name="psc")
                        for kc in range(CB):
                            nc.tensor.matmul(
                                ps[:, :], hnl[:, kc, pb * P:(pb + 1) * P],
                                w_sb["wv"][:, kc, :],
                                start=(kc == 0), stop=(kc == CB - 1))
                        nc.scalar.copy(out=vT_sb[:, pb, :], in_=ps)
                    ks[fi], qs[fi], vs[fi] = k_sb, q_sb, vT_sb

                conv_block(0)
                for f in range(T):
                    xf = xfs[f]
                    k_sb, q_sb, vT_sb = ks[f], qs[f], vs[f]
                    if f + 1 < T:
                        load_x(f + 1)

                    if f + 1 < T:
                        gn_block(f + 1)

                    # ---- scores + softmax (no max-subtraction: |scores| ~ 1) ----
                    att = spat1.tile([P, QB, HW], bf16, tag="att", name="att")
                    den = spat2.tile([P, QB, 2], f32, tag="den", name="den")
                    for qb in range(QB):
                        for half in range(2):
                            psS = psum.tile([P, 512], f32, tag="psc", name="psc")
                            for kc in range(CB):
                                nc.tensor.matmul(
                                    psS[:, :],
                                    q_sb[:, kc, qb * P:(qb + 1) * P],
                                    k_sb[:, kc, half * 512:(half + 1) * 512],
                                    start=(kc == 0), stop=(kc == CB - 1))
                            nc.scalar.activation(
                                out=att[:, qb, half * 512:(half + 1) * 512],
                                in_=psS, func=AF.Exp, scale=SCALE,
                                accum_out=den[:, qb, half:half + 1])
                    rden = spat2.tile([P, QB], f32, tag="rden", name="rden")
                    nc.vector.tensor_reduce(out=rden, in_=den, axis=AX.X, op=ADD)
                    nc.vector.reciprocal(rden, rden)
                    for qb in range(QB):
                        nc.gpsimd.tensor_tensor(
                            out=att[:, qb, :], in0=att[:, qb, :],
                            in1=rden[:, qb:qb + 1].to_broadcast([P, HW]), op=MULT)

                    # ---- transpose att (batched DMA xbar) -> attT[kpix, kb, q] ----
                    attT = spat1.tile([P, KB, HALF], bf16, tag="attT", name="attT", bufs=2)
                    for qb in range(QB):
                        nc.sync.dma_start(
                            out=attT[:, :, qb * P:(qb + 1) * P],
                            in_=att[:, qb, :], transpose=True)

                    # next frame's convs: PE filler under the
                    # att-transpose latency
                    if f + 1 < T:
                        conv_block(f + 1)

                    # ---- hsp = v @ attT  (+bv via softmax-sums-to-1) ----
                    hsp = spat1.tile([P, CB, HALF], bf16, tag="hsp", name="hsp")
                    for cb in range(CB):
                        ps = psum.tile([P, 512], f32, tag="psc", name="psc")
                        for kb in range(KB):
                            nc.tensor.matmul(
                                ps[:, :], vT_sb[:, kb, cb * P:(cb + 1) * P],
                                attT[:, kb, :],
                                start=(kb == 0), stop=(kb == KB - 1))
                        nc.scalar.activation(
                            out=hsp[:, cb, :], in_=ps, func=AF.Identity,
                            bias=bias_sb["bv"][:, cb:cb + 1])

                    # ---- spatio = x + wo @ hsp + bo ; GN_t partial sums ----
                    spatio = spatio_p.tile([P, CB, HALF], bf16, tag="spatio",
                                           name="spatio")
                    sums_t = spat2.tile([P, CB, 2], f32, tag="sums_t", name="sums_t")
                    sqt = spat2.tile([P, 512], f32, tag="sqt", name="sqt")
                    for cb in range(CB):
                        ps = psum.tile([P, 512], f32, tag="psc", name="psc")
                        for kc in range(CB):
                            nc.tensor.matmul(
                                ps[:, :], w_sb["wo"][:, kc, cb * P:(cb + 1) * P],
                                hsp[:, kc, :],
                                start=(kc == 0), stop=(kc == CB - 1))
                        tmpo = spat2.tile([P, 512], f32, tag="tmpo", name="tmpo")
                        nc.scalar.activation(out=tmpo, in_=ps, func=AF.Identity,
                                             bias=bias_sb["bo"][:, cb:cb + 1])
                        nc.gpsimd.tensor_tensor(out=spatio[:, cb, :], in0=tmpo,
                                                in1=xf[:, cb, 0:HALF], op=ADD)
                        nc.vector.tensor_reduce(out=sums_t[:, cb, 0:1],
                                                in_=spatio[:, cb, :],
                                                axis=AX.X, op=ADD)
                        nc.scalar.activation(out=sqt, in_=spatio[:, cb, :],
                                             func=AF.Square,
                                             accum_out=sums_t[:, cb, 1:2])

                    # ---- per-frame pair AllReduce of GN_t sums -> gnt[f] ----
                    ps_gt = ps_aff.tile([P, 512], f32, tag="ps_stat", name="ps_gt")
                    nc.tensor.matmul(ps_gt[0:G, 0:2 * CB], sel4[:, :],
                                     sums_t.rearrange("p j s -> p (j s)"),
                                     start=True, stop=True)
                    g2t = stat4.tile([G, 2], f32, tag="g2t", name="g2t")
                    nc.vector.tensor_reduce(
                        out=g2t,
                        in_=ps_gt[0:G, 0:2 * CB].rearrange("g (j s) -> g s j", s=2),
                        axis=AX.X, op=ADD)
                    bounce_in = dram.tile([G, 2], f32, tag="bnc_in", name="bnc_in")
                    bounce_outs[f] = dram.tile([G, 2], f32, tag="bnc_out",
                                               name="bnc_out")
                    nc.sync.dma_start(out=bounce_in[:], in_=g2t[:])
                    nc.gpsimd.collective_compute(
                        "AllReduce", ADD,
                        replica_groups=[[0, 1], [2, 3], [4, 5], [6, 7]],
                        ins=[bounce_in.opt()], outs=[bounce_outs[f].opt()])
                    spatio_tiles[f] = spatio
                    # frame f-1's post-collective tail (one frame of slack so
                    # the in-order engine queues never wait on the collective)
                    if f > 0:
                        temporal_tail(f - 1)
                if True:
                    temporal_tail(T - 1)

            # ================= temporal phase =================
            # Per pixel-block pb: pack q,k as [P, T, C] and v as [P, C, T],
            # then per-pixel 5x5 attention via batched mul+reduce on DVE.
            with tc.tile_pool(name="temp5", bufs=T) as temp5, \
                 tc.tile_pool(name="temp2", bufs=2) as temp2, \
                 tc.tile_pool(name="temp4", bufs=4) as temp4:
                htp_b = []
                for t in range(T):
                    htp_b.append(temp5.tile([P, QB, C], bf16, tag="htp_b",
                                            name="htp_b"))
                for pb in range(QB):
                    qp = temp2.tile([P, T, C], bf16, tag="q5P", name="q5P")
                    kp = temp2.tile([P, T, C], bf16, tag="k5P", name="k5P")
                    vp = temp2.tile([P, T, C], bf16, tag="v5P", name="v5P")
                    for t in range(T):
                        for w_nm, dst in (("wqt", qp[:, t, :]), ("wkt", kp[:, t, :]),
                                          ("wvt", vp[:, t, :])):
                            ps = psum.tile([P, 512], f32, tag="psc", name="psc")
                            for kc in range(CB):
                                nc.tensor.matmul(
                                    ps[:, :], gnt[t][:, kc, pb * P:(pb + 1) * P],
                                    w_sb[w_nm][:, kc, :],
                                    start=(kc == 0), stop=(kc == CB - 1))
                            nc.scalar.copy(out=dst, in_=ps)

                    # scores sc[t,s] = sum_c q[t,c]k[s,c] (+ bqt.k[s], softmax-
                    # invariant terms dropped); SCALE folded into the exp.
                    # DVE does the batched muls; ACT reduces via accum_out.
                    sc = temp4.tile([P, T, T], f32, tag="sc", name="sc")
                    junkb = temp4.tile([P, C], bf16, tag="junkb", name="junkb")
                    with nc.allow_low_precision("bf16 score rounding ok"):
                        for t in range(T):
                            mbuf = temp4.tile([P, T, C], bf16, tag="mbuf",
                                              name="mbuf", bufs=3)
                            nc.vector.tensor_tensor(
                                out=mbuf, in0=kp,
                                in1=qp[:, t, :].unsqueeze(1).to_broadcast([P, T, C]),
                                op=MULT)
                            if (pb + t) % 4 == 3:
                                nc.vector.tensor_reduce(
                                    out=sc[:, t, :], in_=mbuf, axis=AX.X, op=ADD)
                            else:
                                for s in range(T):
                                    nc.scalar.activation(
                                        out=junkb, in_=mbuf[:, s, :], func=AF.Copy,
                                        accum_out=sc[:, t, :][:, s:s + 1])
                        mbufd = temp4.tile([P, T, C], bf16, tag="mbuf", name="mbufd", bufs=3)
                        nc.vector.tensor_tensor(
                            out=mbufd, in0=kp,
                            in1=bqt_bc.unsqueeze(1).to_broadcast([P, T, C]),
                            op=MULT)
                        dotk = temp4.tile([P, T], f32, tag="dotk", name="dotk")
                        nc.vector.tensor_reduce(out=dotk, in_=mbufd, axis=AX.X,
                                                op=ADD)
                        nc.vector.tensor_tensor(
                            out=sc, in0=sc,
                            in1=dotk.unsqueeze(1).to_broadcast([P, T, T]), op=ADD)
                    e5 = temp4.tile([P, T, T], f32, tag="e5", name="e5")
                    nc.scalar.activation(out=e5, in_=sc, func=AF.Exp, scale=SCALE)
                    den5 = temp4.tile([P, T], f32, tag="den5", name="den5")
                    nc.vector.tensor_reduce(out=den5, in_=e5, axis=AX.X, op=ADD)
                    rden5 = temp4.tile([P, T], f32, tag="rden5", name="rden5")
                    nc.vector.reciprocal(rden5, den5)
                    a5 = temp4.tile([P, T, T], bf16, tag="a5", name="a5")
                    with nc.allow_low_precision("bf16 att rounding ok"):
                        nc.vector.tensor_tensor(
                            out=a5, in0=e5,
                            in1=rden5.unsqueeze(2).to_broadcast([P, T, T]), op=MULT)

                    # htp[t] = sum_s att[t,s] * v5[s]: batched mul + add tree
                    with nc.allow_low_precision("bf16 htp rounding ok"):
                        for t in range(T):
                            eng = nc.gpsimd if (pb + t) % 2 == 1 else nc.vector
                            mb2 = temp4.tile([P, T, C], bf16, tag="mbuf2",
                                             name="mbuf2", bufs=2)
                            nc.vector.tensor_tensor(
                                out=mb2, in0=vp,
                                in1=a5[:, t, :].unsqueeze(2).to_broadcast([P, T, C]),
                                op=MULT)
                            h01 = temp4.tile([P, C], bf16, tag="h01", name="h01")
                            h23 = temp4.tile([P, C], bf16, tag="h23", name="h23")
                            eng.tensor_tensor(out=h01, in0=mb2[:, 0, :],
                                              in1=mb2[:, 1, :], op=ADD)
                            eng.tensor_tensor(out=h23, in0=mb2[:, 2, :],
                                              in1=mb2[:, 3, :], op=ADD)
                            eng.tensor_tensor(out=h01, in0=h01, in1=h23, op=ADD)
                            eng.tensor_tensor(out=htp_b[t][:, pb, :], in0=h01,
                                              in1=mb2[:, 4, :], op=ADD)

                for t in range(T):
                    htpT = temp2.tile([P, CB, HALF], bf16, tag="htpT", name="htpT")
                    for pb in range(QB):
                        nc.sync.dma_start(
                            out=htpT[:, :, pb * P:(pb + 1) * P],
                            in_=htp_b[t][:, pb, :], transpose=True)
                    # out = x + wot @ htpT + bot_eff
                    xh = temp2.tile([P, CB, HALF], f32, tag="xh", name="xh")
                    nc.sync.dma_start(
                        out=xh,
                        in_=x_d[t][:, 0:HALF].rearrange("(p j) hw -> p j hw", p=P))
                    out_sb = temp2.tile([P, CB, HALF], f32, tag="out_sb",
                                        name="out_sb")
                    for cb in range(CB):
                        ps = psum.tile([P, 512], f32, tag="psc", name="psc")
                        for kc in range(CB):
                            nc.tensor.matmul(
                                ps[:, :], w_sb["wot"][:, kc, cb * P:(cb + 1) * P],
                                htpT[:, kc, :],
                                start=(kc == 0), stop=(kc == CB - 1))
                        tmpo2 = temp2.tile([P, 512], f32, tag="tmpo2", name="tmpo2")
                        nc.vector.tensor_scalar_add(out=tmpo2, in0=ps,
                                                    scalar1=bias_sb["bot"][:, cb:cb + 1])
                        nc.gpsimd.tensor_tensor(out=out_sb[:, cb, :], in0=tmpo2,
                                                in1=xh[:, cb, :], op=ADD)
                    nc.sync.dma_start(
                        out=out_d[t].rearrange("(p j) hw -> p j hw", p=P),
                        in_=out_sb)

    nc.compile()
    return nc


# storage column s holds natural channel 4*(s % 128) + s // 128
_COL_PERM = np.array([4 * (s % P) + s // P for s in range(C)])


def _prepare_in_maps(inputs):
    import ml_dtypes
    x = np.asarray(inputs["x"], np.float32).reshape(B * T, C, HW)
    sel4 = np.zeros((P, G), np.float32)
    for p in range(P):
        sel4[p, p // 4] = 1.0
    bcast4 = sel4.T.copy()
    wT = {}
    for nm in ["wq", "wk", "wv", "wo", "wqt", "wkt", "wvt", "wot"]:
        w = np.asarray(inputs[nm], np.float32)   # [out, in]
        wt = w.T[:, _COL_PERM]                   # [in, out_perm]
        wT[nm] = np.ascontiguousarray(wt).astype(ml_dtypes.bfloat16)
    bot_eff = (np.asarray(inputs["bot"], np.float64)
               + np.asarray(inputs["wot"], np.float64)
               @ np.asarray(inputs["bvt"], np.float64)).astype(np.float32)
    common = {nm + "T": wT[nm] for nm in wT}
    for nm in ["bq", "bk", "bv", "bo"]:
        common[nm] = np.asarray(inputs[nm], np.float32)
    common["bot"] = bot_eff
    # bqt multiplies k-columns, which carry the permuted channel order
    common["bqt"] = np.asarray(inputs["bqt"], np.float32)[_COL_PERM] \
        .astype(ml_dtypes.bfloat16)
    for nm in ["gamma_s", "beta_s", "gamma_t", "beta_t"]:
        common[nm] = np.asarray(inputs[nm], np.float32)
    common["sel4"] = sel4
    common["bcast4"] = bcast4

    in_maps = []
    for v in range(B):
        xv = x[v * T:(v + 1) * T]
        for h in range(2):
            if h == 0:
                xc = xv
            else:
                xc = np.concatenate([xv[..., HALF:], xv[..., :HALF]], axis=-1)
            m = dict(common)
            m["x"] = np.ascontiguousarray(xc)
            in_maps.append(m)
    return in_maps


def _run(inputs, trace=False):
    from concourse import bass_utils
    if "nc" not in _CACHE:
        _CACHE["nc"] = _build()
    nc = _CACHE["nc"]
    in_maps = _prepare_in_maps(inputs)
    if trace:
        try:
            from antenv.axon_hooks import get_axon_ntff_profile_hook  # noqa: F401
        except ModuleNotFoundError:
            trace = False
    res = bass_utils.run_bass_kernel_spmd(nc, in_maps, core_ids=list(range(8)),
                                          trace=trace)
    out = np.empty((B * T, C, HW), np.float32)
    for v in range(B):
        for h in range(2):
            o = res.results[2 * v + h]["out"]  # [T, C, HALF]
            if h == 0:
                out[v * T:(v + 1) * T, :, :HALF] = o
            else:
                out[v * T:(v + 1) * T, :, HALF:] = o
    return out.reshape(B * T, C, 32, 32), res


def kernel(**inputs) -> np.ndarray:
    out, _ = _run(inputs, trace=False)
    return out



# revision 14
# speedup vs baseline: 1.5666x; 1.5666x over previous
"""Trainium2 Bass kernel for nn_AttnBlock_Spatio_Temporal (B=4,T=5,C=512,H=W=32).

Distribution: 8 cores = (video b in 0..3) x (pixel-half h in 0..1); host rolls
the HW axis per core so its own 512 pixels come first. All heavy matmuls run
in fp8e4 DoubleRow (K=256/instruction, fp32 accumulate); weights host-scaled
x64 to stay in fp8 normal range, unscaled in the PSUM->SBUF epilogues.

Spatial attention is computed TRANSPOSED (scoresT[k,q]) so the softmax key
axis lands on partitions: exp goes straight to fp8 eT tiles, the denominator
is a fp8 ones-matmul on PE (result replicated across all partitions), and
1/den is folded into the hsp epilogue. No DMA transposes, no separate
normalize pass.

GroupNorm group stats use a pre-broadcast selector matmul (sel (x) ones4):
group sums land on all 128 partitions, so the temporal-GN pair-AllReduce
bounces [128,2] and the post-collective tail is pure per-partition小 ops
(no PE op waits on the collective). rsqrt = exp(-0.5*ln(var+eps)) keeps every
ACT func in one activation table (no reload churn).

Temporal attention: per-pixel 5x5 scores via fused tensor_tensor_reduce
(one op per (t,s) pair), issued incrementally during the spatial phase two
frames behind; apply is a 5-op scalar_tensor_tensor chain per (pb,t) split
across DVE/GpSimd; wot runs bf16 after bf16 DMA transposes.
"""
import numpy as np

B, T, C, HW = 4, 5, 512, 1024
G = 32
EPS = 1e-6
P = 128
CB = C // P          # 4 channel blocks
HALF = HW // 2       # 512 own pixels
KB = HW // P         # 8 key-pixel blocks
QB = HALF // P       # 4 query/pixel blocks
SCALE = float(C) ** -0.5
CNT = 16384.0        # per-group element count (16ch*1024px)
CNT_H = 2048.0       # per-(partition,cb) element count for GN_t halves... (unused)
WS = 64.0            # fp8 weight scale

_CACHE = {}


def _build():
    import concourse.bacc as bacc
    import concourse.tile as tile
    import concourse.mybir as mybir

    f32 = mybir.dt.float32
    bf16 = mybir.dt.bfloat16
    fp8 = mybir.dt.float8e4
    MULT = mybir.AluOpType.mult
    ADD = mybir.AluOpType.add
    SUB = mybir.AluOpType.subtract
    AF = mybir.ActivationFunctionType
    AX = mybir.AxisListType
    DR = mybir.MatmulPerfMode.DoubleRow

    nc = bacc.Bacc("TRN2", target_bir_lowering=False, debug=False, num_devices=8)

    x_d = nc.dram_tensor("x", [T, C, HW], f32, kind="ExternalInput").ap()
    w8_names = ["wq", "wk", "wv", "wo", "wqt", "wkt", "wvt"]
    w_d = {nm: nc.dram_tensor(nm + "T", [C, C], fp8, kind="ExternalInput").ap()
           for nm in w8_names}
    wot_d = nc.dram_tensor("wotT", [C, C], bf16, kind="ExternalInput").ap()
    b_d = {nm: nc.dram_tensor(nm, [C], f32, kind="ExternalInput").ap()
           for nm in ["bq", "bk", "bo", "bot"]}
    bqt_d = nc.dram_tensor("bqt", [C], bf16, kind="ExternalInput").ap()
    g_d = {nm: nc.dram_tensor(nm, [C], f32, kind="ExternalInput").ap()
           for nm in ["gamma_s", "beta_s", "gamma_t", "beta_t"]}
    selbc_d = nc.dram_tensor("selbc", [P, P], bf16, kind="ExternalInput").ap()
    out_d = nc.dram_tensor("out", [T, C, HALF], bf16, kind="ExternalOutput").ap()

    def cpart(ap_1d):  # [C] dram -> [128, CB] tile order (c = 4p + j)
        return ap_1d.rearrange("(p j) -> p j", p=P)

    with tile.TileContext(nc) as tc:
        with tc.tile_pool(name="consts", bufs=1) as consts, \
             tc.tile_pool(name="stat4", bufs=4) as stat4, \
             tc.tile_pool(name="xfp", bufs=2) as xfp, \
             tc.tile_pool(name="xhp", bufs=T) as xhp, \
             tc.tile_pool(name="hnp", bufs=1) as hnp, \
             tc.tile_pool(name="kqp", bufs=1) as kqp, \
             tc.tile_pool(name="spp", bufs=3) as spp, \
             tc.tile_pool(name="gntp", bufs=3) as gntp, \
             tc.tile_pool(name="tp2", bufs=2) as tp2, \
             tc.tile_pool(name="psA", bufs=3, space="PSUM") as psA, \
             tc.tile_pool(name="psB", bufs=2, space="PSUM") as psB, \
             tc.tile_pool(name="dram", bufs=T, space="DRAM") as dram:

            # ---------------- constants ----------------
            w_sb = {}
            for nm in w8_names:
                w_sb[nm] = consts.tile([P, CB, C], fp8, tag="w_" + nm,
                                       name="w_" + nm)
                nc.gpsimd.dma_start(
                    out=w_sb[nm],
                    in_=w_d[nm].rearrange("(p kc) co -> p kc co", p=P))
            wot_sb = consts.tile([P, CB, C], bf16, tag="w_wot", name="w_wot")
            nc.gpsimd.dma_start(
                out=wot_sb, in_=wot_d.rearrange("(p kc) co -> p kc co", p=P))
            bias_sb = {}
            for nm in ["bq", "bk", "bo", "bot"]:
                bias_sb[nm] = consts.tile([P, CB], f32, tag="b_" + nm,
                                          name="b_" + nm)
                nc.gpsimd.dma_start(out=bias_sb[nm], in_=cpart(b_d[nm]))
            gam_sb = {}
            for nm in ["gamma_s", "beta_s", "gamma_t", "beta_t"]:
                gam_sb[nm] = consts.tile([P, CB], f32, tag="g_" + nm,
                                         name="g_" + nm)
                nc.gpsimd.dma_start(out=gam_sb[nm], in_=cpart(g_d[nm]))
            bqt_bc = consts.tile([P, C], bf16, tag="bqt_bc", name="bqt_bc")
            nc.gpsimd.dma_start(out=bqt_bc,
                                in_=bqt_d.unsqueeze(0).to_broadcast([P, C]))
            selbc = consts.tile([P, P], bf16, tag="selbc", name="selbc")
            nc.gpsimd.dma_start(out=selbc, in_=selbc_d)
            ones8 = consts.tile([P, 2, P], fp8, tag="ones8", name="ones8")
            nc.vector.memset(ones8, 1.0)
            eps_t = consts.tile([P, 1], f32, tag="eps_t", name="eps_t")
            nc.vector.memset(eps_t, EPS)
            # temporal q/k/v activations, all frames resident
            qp_all = consts.tile([P, QB, T, C], bf16, tag="qp_all", name="qp_all")
            kp_all = consts.tile([P, QB, T, C], bf16, tag="kp_all", name="kp_all")
            vp_all = consts.tile([P, QB, T, C], bf16, tag="vp_all", name="vp_all")
            sc5 = consts.tile([P, QB, T, T], f32, tag="sc5", name="sc5")
            dotk5 = consts.tile([P, QB, T], f32, tag="dotk5", name="dotk5")

            xfs = [None] * T
            xhalfs = [None] * T
            hns = [None] * T
            scale_s = [None] * T
            shift_s = [None] * T
            spatio_tiles = [None] * T
            gnt = [None] * T
            bounce_outs = [None] * T

            def load_x(fi):
                xf = xfp.tile([P, CB, HW], f32, tag="xf", name="xf%d" % fi)
                nc.sync.dma_start(
                    out=xf, in_=x_d[fi].rearrange("(p j) hw -> p j hw", p=P))
                xfs[fi] = xf

            def gn_stats(fi):
                """bn_stats/aggr over xf -> per-partition (sum,sumsq) bf16."""
                xf = xfs[fi]
                st = stat4.tile([P, 2 * CB, 6], f32, tag="st", name="st%d" % fi)
                for j in range(CB):
                    for h in range(2):
                        nc.vector.bn_stats(
                            out=st[:, 2 * j + h, :],
                            in_=xf[:, j, h * 512:(h + 1) * 512])
                mv = stat4.tile([P, 2], f32, tag="mv", name="mv%d" % fi)
                nc.vector.bn_aggr(out=mv, in_=st)
                ss = stat4.tile([P, 2], bf16, tag="ss", name="ss%d" % fi)
                # ss[:,0] = sum = mean*CNT/4... per-partition count is 4096
                with nc.allow_low_precision("bf16 GN stats"):
                    nc.vector.tensor_scalar(out=ss[:, 0:1], in0=mv[:, 0:1],
                                            scalar1=4096.0, scalar2=0.0,
                                            op0=MULT, op1=ADD)
                    # sumsq = (var + mean^2) * 4096
                    m2 = stat4.tile([P, 1], f32, tag="m2", name="m2_%d" % fi)
                    nc.vector.tensor_tensor(out=m2, in0=mv[:, 0:1],
                                            in1=mv[:, 0:1], op=MULT)
                    nc.vector.tensor_tensor(out=m2, in0=mv[:, 1:2],
                                            in1=m2, op=ADD)
                    nc.vector.tensor_scalar(out=ss[:, 1:2], in0=m2,
                                            scalar1=4096.0, scalar2=0.0,
                                            op0=MULT, op1=ADD)
                return ss

            def gn_affine(fi):
                ss = gn_stats(fi)
                psg = psB.tile([P, 512], f32, tag="psb", name="psg%d" % fi)
                nc.tensor.matmul(psg[:, 0:2], selbc[:, :], ss[:, :],
                                 start=True, stop=True)
                scl = stat4.tile([P, CB], f32, tag="scl", name="scl%d" % fi)
                shf = stat4.tile([P, CB], f32, tag="shf", name="shf%d" % fi)
                mz = stat4.tile([P, 2], f32, tag="mz", name="mzs%d" % fi)
                nc.vector.tensor_scalar(out=mz, in0=psg[:, 0:2],
                                        scalar1=1.0 / CNT, scalar2=0.0,
                                        op0=MULT, op1=ADD)
                vr = stat4.tile([P, 1], f32, tag="vr", name="vrs%d" % fi)
                nc.vector.tensor_tensor(out=vr, in0=mz[:, 0:1], in1=mz[:, 0:1],
                                        op=MULT)
                nc.vector.tensor_tensor(out=vr, in0=mz[:, 1:2], in1=vr, op=SUB)
                nc.scalar.activation(out=vr, in_=vr, func=AF.Ln, bias=eps_t,
                                     scale=1.0)
                nc.scalar.activation(out=vr, in_=vr, func=AF.Exp, scale=-0.5)
                nc.vector.tensor_scalar_mul(out=scl, in0=gam_sb["gamma_s"],
                                            scalar1=vr)
                nmr = stat4.tile([P, 1], f32, tag="nmr", name="nmrs%d" % fi)
                nc.vector.tensor_scalar(out=nmr, in0=mz[:, 0:1], scalar1=vr,
                                        scalar2=-1.0, op0=MULT, op1=MULT)
                nc.vector.scalar_tensor_tensor(out=shf, in0=gam_sb["gamma_s"],
                                               scalar=nmr, in1=gam_sb["beta_s"],
                                               op0=MULT, op1=ADD)
                scale_s[fi], shift_s[fi] = scl, shf

            def gn_apply(fi):
                hn = hnp.tile([P, CB, HW], fp8, tag="hn", name="hn%d" % fi)
                with nc.allow_low_precision("fp8 hn"):
                    for j in range(CB):
                        nc.scalar.activation(
                            out=hn[:, j, :], in_=xfs[fi][:, j, :], func=AF.Identity,
                            bias=shift_s[fi][:, j:j + 1],
                            scale=scale_s[fi][:, j:j + 1])
                hns[fi] = hn

            def xhalf_cast(fi):
                xh = xhp.tile([P, CB, HALF], bf16, tag="xh", name="xhf%d" % fi)
                with nc.allow_low_precision("bf16 x residual"):
                    nc.scalar.activation(out=xh, in_=xfs[fi][:, :, 0:HALF],
                                         func=AF.Copy, scale=1.0)
                xhalfs[fi] = xh

            def dr_conv(ps, w_tile, rhs_tile, out_slice=None):
                """psum[:, sl] += sum_cb w.T @ rhs via 2 DoubleRow matmuls."""
                for u in range(2):
                    nc.tensor.matmul(ps, w_tile[:, 2 * u:2 * u + 2, :],
                                     rhs_tile[:, 2 * u:2 * u + 2, :],
                                     start=(u == 0), stop=(u == 1),
                                     perf_mode=DR)

            # ---------------- spatial frame body ----------------
            def conv_k(fi):
                hn = hns[fi]
                k_sb = kqp.tile([P, CB, HW], fp8, tag="k_sb", name="k%d" % fi)
                # per jo one [P,1024] psum = full HW for that output chunk
                for jo in range(CB):
                    ps = psA.tile([P, 1024], f32, tag="ps",
                                  name="psk%d_%d" % (fi, jo))
                    for half in range(2):
                        for u in range(2):
                            nc.tensor.matmul(
                                ps[:, half * 512:(half + 1) * 512],
                                w_sb["wk"][:, 2 * u:2 * u + 2,
                                           jo * P:(jo + 1) * P],
                                hn[:, 2 * u:2 * u + 2,
                                   half * 512:(half + 1) * 512],
                                start=(u == 0), stop=(u == 1), perf_mode=DR)
                    with nc.allow_low_precision("fp8 k"):
                        nc.scalar.activation(out=k_sb[:, jo, :], in_=ps,
                                             func=AF.Identity, scale=1.0 / WS,
                                             bias=bias_sb["bk"][:, jo:jo + 1])
                return k_sb

            def conv_q(fi):
                hn = hns[fi]
                q_sb = kqp.tile([P, CB, HALF], fp8, tag="q_sb", name="q%d" % fi)
                for jo in range(0, CB, 2):
                    ps = psA.tile([P, 1024], f32, tag="ps",
                                  name="psq%d_%d" % (fi, jo))
                    for dj in range(2):
                        for u in range(2):
                            nc.tensor.matmul(
                                ps[:, dj * 512:(dj + 1) * 512],
                                w_sb["wq"][:, 2 * u:2 * u + 2,
                                           (jo + dj) * P:(jo + dj + 1) * P],
                                hn[:, 2 * u:2 * u + 2, 0:HALF],
                                start=(u == 0), stop=(u == 1), perf_mode=DR)
                    with nc.allow_low_precision("fp8 q"):
                        for dj in range(2):
                            nc.scalar.activation(
                                out=q_sb[:, jo + dj, :],
                                in_=ps[:, dj * 512:(dj + 1) * 512],
                                func=AF.Identity, scale=1.0 / WS,
                                bias=bias_sb["bq"][:, jo + dj:jo + dj + 1])
                return q_sb

            def scores_exp(fi, k_sb, q_sb):
                eT = kqp.tile([P, KB, HALF], fp8, tag="eT", name="eT%d" % fi)
                for kb in range(0, KB, 2):
                    ps = psA.tile([P, 1024], f32, tag="ps",
                                  name="pss%d_%d" % (fi, kb))
                    for dk in range(2):
                        for u in range(2):
                            nc.tensor.matmul(
                                ps[:, dk * 512:(dk + 1) * 512],
                                k_sb[:, 2 * u:2 * u + 2,
                                     (kb + dk) * P:(kb + dk + 1) * P],
                                q_sb[:, 2 * u:2 * u + 2, :],
                                start=(u == 0), stop=(u == 1), perf_mode=DR)
                    with nc.allow_low_precision("fp8 eT"):
                        nc.scalar.activation(
                            out=eT[:, kb:kb + 2, :],
                            in_=ps.rearrange("p (d q) -> p d q", d=2),
                            func=AF.Exp, scale=SCALE)
                return eT

            def den_recip(fi, eT):
                ps = psB.tile([P, 512], f32, tag="psb", name="psd%d" % fi)
                for u in range(KB // 2):
                    nc.tensor.matmul(ps[:, :], ones8[:, :, :],
                                     eT[:, 2 * u:2 * u + 2, :],
                                     start=(u == 0), stop=(u == KB // 2 - 1),
                                     perf_mode=DR)
                rden = kqp.tile([P, HALF], bf16, tag="rden", name="rden%d" % fi)
                with nc.allow_low_precision("bf16 rden"):
                    nc.vector.reciprocal(rden, ps)
                return rden

            def conv_v(fi):
                hn = hns[fi]
                vT = kqp.tile([P, KB, C], fp8, tag="vT", name="vT%d" % fi)
                for pb in range(0, KB, 2):
                    for dp in range(2):
                        ps = psB.tile([P, 512], f32, tag="psb",
                                      name="psv%d_%d" % (fi, pb + dp))
                        for u in range(2):
                            nc.tensor.matmul(
                                ps[:, :],
                                hn[:, 2 * u:2 * u + 2,
                                   (pb + dp) * P:(pb + dp + 1) * P],
                                w_sb["wv"][:, 2 * u:2 * u + 2, :],
                                start=(u == 0), stop=(u == 1), perf_mode=DR)
                        with nc.allow_low_precision("fp8 vT"):
                            nc.scalar.activation(out=vT[:, pb + dp, :], in_=ps,
                                                 func=AF.Copy, scale=1.0 / WS)
                return vT

            def hsp_wo_spatio(fi, vT, eT, rden):
                hsp = kqp.tile([P, CB, HALF], fp8, tag="hsp", name="hsp%d" % fi)
                for cb in range(0, CB, 2):
                    ps = psA.tile([P, 1024], f32, tag="ps",
                                  name="psh%d_%d" % (fi, cb))
                    for dc in range(2):
                        for u in range(KB // 2):
                            nc.tensor.matmul(
                                ps[:, dc * 512:(dc + 1) * 512],
                                vT[:, 2 * u:2 * u + 2,
                                   (cb + dc) * P:(cb + dc + 1) * P],
                                eT[:, 2 * u:2 * u + 2, :],
                                start=(u == 0), stop=(u == KB // 2 - 1),
                                perf_mode=DR)
                    with nc.allow_low_precision("fp8 hsp"):
                        nc.vector.tensor_tensor(
                            out=hsp[:, cb:cb + 2, :],
                            in0=ps.rearrange("p (d q) -> p d q", d=2),
                            in1=rden.unsqueeze(1).to_broadcast([P, 2, HALF]),
                            op=MULT)
                # wo conv + residual + GN_t stats
                spatio = spp.tile([P, CB, HALF], bf16, tag="spatio",
                                  name="spat%d" % fi)
                tmpo = tp2.tile([P, CB, HALF], bf16, tag="tmpo",
                                name="tmpo%d" % fi, bufs=1)
                for cb in range(0, CB, 2):
                    ps = psA.tile([P, 1024], f32, tag="ps",
                                  name="psw%d_%d" % (fi, cb))
                    for dc in range(2):
                        for u in range(2):
                            nc.tensor.matmul(
                                ps[:, dc * 512:(dc + 1) * 512],
                                w_sb["wo"][:, 2 * u:2 * u + 2,
                                           (cb + dc) * P:(cb + dc + 1) * P],
                                hsp[:, 2 * u:2 * u + 2, :],
                                start=(u == 0), stop=(u == 1), perf_mode=DR)
                    with nc.allow_low_precision("bf16 tmpo"):
                        for dc in range(2):
                            nc.scalar.activation(
                                out=tmpo[:, cb + dc, :],
                                in_=ps[:, dc * 512:(dc + 1) * 512],
                                func=AF.Identity, scale=1.0 / WS,
                                bias=bias_sb["bo"][:, cb + dc:cb + dc + 1])
                with nc.allow_low_precision("bf16 spatio"):
                    nc.gpsimd.tensor_tensor(out=spatio, in0=tmpo,
                                            in1=xhalfs[fi], op=ADD)
                spatio_tiles[fi] = spatio
                return spatio

            def gnt_stats_collective(fi, spatio):
                st = stat4.tile([P, CB, 6], f32, tag="stt", name="stt%d" % fi)
                for j in range(CB):
                    nc.vector.bn_stats(out=st[:, j, :], in_=spatio[:, j, :])
                mv = stat4.tile([P, 2], f32, tag="mvt", name="mvt%d" % fi)
                nc.vector.bn_aggr(out=mv, in_=st)
                ss = stat4.tile([P, 2], bf16, tag="sst", name="sst%d" % fi)
                with nc.allow_low_precision("bf16 GN_t stats"):
                    nc.vector.tensor_scalar(out=ss[:, 0:1], in0=mv[:, 0:1],
                                            scalar1=2048.0, scalar2=0.0,
                                            op0=MULT, op1=ADD)
                    m2 = stat4.tile([P, 1], f32, tag="m2t", name="m2t%d" % fi)
                    nc.vector.tensor_tensor(out=m2, in0=mv[:, 0:1],
                                            in1=mv[:, 0:1], op=MULT)
                    nc.vector.tensor_tensor(out=m2, in0=mv[:, 1:2],
                                            in1=m2, op=ADD)
                    nc.vector.tensor_scalar(out=ss[:, 1:2], in0=m2,
                                            scalar1=2048.0, scalar2=0.0,
                                            op0=MULT, op1=ADD)
                psg = psB.tile([P, 512], f32, tag="psb", name="psgt%d" % fi)
                nc.tensor.matmul(psg[:, 0:2], selbc[:, :], ss[:, :],
                                 start=True, stop=True)
                g2 = stat4.tile([P, 2], f32, tag="g2", name="g2t%d" % fi)
                nc.vector.tensor_copy(out=g2, in_=psg[:, 0:2])
                bounce_in = dram.tile([P, 2], f32, tag="bnc_in", name="bi%d" % fi)
                bounce_outs[fi] = dram.tile([P, 2], f32, tag="bnc_out",
                                            name="bo%d" % fi)
                nc.gpsimd.dma_start(out=bounce_in[:], in_=g2[:])
                nc.gpsimd.collective_compute(
                    "AllReduce", ADD,
                    replica_groups=[[0, 1], [2, 3], [4, 5], [6, 7]],
                    ins=[bounce_in.opt()], outs=[bounce_outs[fi].opt()])

            def tail(fi):
                """post-collective: finalize GN_t affine, apply -> gnt fp8."""
                g2 = stat4.tile([P, 2], f32, tag="g2r", name="g2r%d" % fi)
                nc.gpsimd.dma_start(out=g2[:], in_=bounce_outs[fi][:])
                mz = stat4.tile([P, 2], f32, tag="mzt", name="mzt%d" % fi)
                nc.vector.tensor_scalar(out=mz, in0=g2, scalar1=1.0 / CNT,
                                        scalar2=0.0, op0=MULT, op1=ADD)
                vr = stat4.tile([P, 1], f32, tag="vrt", name="vrt%d" % fi)
                nc.vector.tensor_tensor(out=vr, in0=mz[:, 0:1], in1=mz[:, 0:1],
                                        op=MULT)
                nc.vector.tensor_tensor(out=vr, in0=mz[:, 1:2], in1=vr, op=SUB)
                nc.scalar.activation(out=vr, in_=vr, func=AF.Ln, bias=eps_t,
                                     scale=1.0)
                nc.scalar.activation(out=vr, in_=vr, func=AF.Exp, scale=-0.5)
                scl = stat4.tile([P, CB], f32, tag="sclt", name="sclt%d" % fi)
                shf = stat4.tile([P, CB], f32, tag="shft", name="shft%d" % fi)
                nc.vector.tensor_scalar_mul(out=scl, in0=gam_sb["gamma_t"],
                                            scalar1=vr)
                nmr = stat4.tile([P, 1], f32, tag="nmrt", name="nmrt%d" % fi)
                nc.vector.tensor_scalar(out=nmr, in0=mz[:, 0:1], scalar1=vr,
                                        scalar2=-1.0, op0=MULT, op1=MULT)
                nc.vector.scalar_tensor_tensor(out=shf, in0=gam_sb["gamma_t"],
                                               scalar=nmr, in1=gam_sb["beta_t"],
                                               op0=MULT, op1=ADD)
                g = gntp.tile([P, CB, HALF], fp8, tag="gnt", name="gnt%d" % fi)
                with nc.allow_low_precision("fp8 gnt"):
                    for j in range(CB):
                        if j % 2 == 0:
                            nc.vector.tensor_scalar(
                                out=g[:, j, :], in0=spatio_tiles[fi][:, j, :],
                                scalar1=scl[:, j:j + 1], scalar2=shf[:, j:j + 1],
                                op0=MULT, op1=ADD)
                        else:
                            nc.scalar.activation(
                                out=g[:, j, :], in_=spatio_tiles[fi][:, j, :],
                                func=AF.Identity, scale=scl[:, j:j + 1],
                                bias=shf[:, j:j + 1])
                gnt[fi] = g

            def tconvs(fi):
                """temporal q/k/v convs for frame fi (pixel-major out)."""
                for pb in range(QB):
                    for w_nm, dst in (("wqt", qp_all), ("wkt", kp_all),
                                      ("wvt", vp_all)):
                        ps = psB.tile([P, 512], f32, tag="psb",
                                      name="pst%s%d_%d" % (w_nm, fi, pb))
                        for u in range(2):
                            nc.tensor.matmul(
                                ps[:, :],
                                gnt[fi][:, 2 * u:2 * u + 2,
                                        pb * P:(pb + 1) * P],
                                w_sb[w_nm][:, 2 * u:2 * u + 2, :],
                                start=(u == 0), stop=(u == 1), perf_mode=DR)
                        with nc.allow_low_precision("bf16 qkv_t"):
                            nc.scalar.activation(
                                out=dst[:, pb, fi, :], in_=ps, func=AF.Copy,
                                scale=1.0 / WS)

            def dotk(fi):
                """dotk5[:, pb, fi] = bqt . kp[fi] (score bias term)."""
                for pb in range(QB):
                    eng = nc.vector
                    junk = tp2.tile([P, C], bf16, tag="junk",
                                    name="junkd%d_%d" % (fi, pb), bufs=2)
                    with nc.allow_low_precision("bf16 dotk"):
                        eng.scalar_tensor_tensor(
                            out=junk, in0=kp_all[:, pb, fi, :], scalar=1.0,
                            in1=bqt_bc, op0=MULT, op1=MULT,
                            accum_out=dotk5[:, pb, fi:fi + 1])

            _pair_toggle = [0]

            def pairs_for(pairs):
                """temporal score TTR for given (t,s) pairs, all pbs."""
                for (t, s) in pairs:
                    for pb in range(QB):
                        _pair_toggle[0] ^= 1
                        eng = nc.vector
                        junk = tp2.tile([P, C], bf16, tag="junk",
                                        name="junkp%d_%d_%d" % (t, s, pb), bufs=2)
                        with nc.allow_low_precision("bf16 sc"):
                            eng.scalar_tensor_tensor(
                                out=junk, in0=qp_all[:, pb, t, :], scalar=1.0,
                                in1=kp_all[:, pb, s, :], op0=MULT, op1=MULT,
                                accum_out=sc5[:, pb, t, s:s + 1])

            # ================= spatial phase =================
            load_x(0)
            gn_affine(0)
            gn_apply(0)
            for f in range(T):
                if f + 1 < T:
                    load_x(f + 1)
                k_sb = conv_k(f)
                q_sb = conv_q(f)
                if f + 1 < T:
                    gn_affine(f + 1)
                eT = scores_exp(f, k_sb, q_sb)
                rden = den_recip(f, eT)
                vT = conv_v(f)
                if f + 1 < T:
                    gn_apply(f + 1)
                xhalf_cast(f)
                hsp_wo_spatio(f, vT, eT, rden)
                gnt_stats_collective(f, spatio_tiles[f])
                if f >= 2:
                    fi = f - 2
                    tail(fi)
                    tconvs(fi)
                    dotk(fi)
                    pairs_for([(t, s) for t in range(fi + 1)
                               for s in range(fi + 1) if max(t, s) == fi])

            # ================= temporal phase =================
            for fi in (T - 2, T - 1):
                tail(fi)
                tconvs(fi)
                dotk(fi)
                pairs_for([(t, s) for t in range(fi + 1)
                           for s in range(fi + 1) if max(t, s) == fi])

            # softmax over s (scores + dotk broadcast over t)
            e5 = consts.tile([P, QB, T, T], f32, tag="e5", name="e5")
            scadj = stat4.tile([P, QB, T, T], f32, tag="scadj", name="scadj")
            nc.vector.tensor_tensor(
                out=scadj, in0=sc5,
                in1=dotk5.unsqueeze(2).to_broadcast([P, QB, T, T]), op=ADD)
            e5f = stat4.tile([P, QB, T, T], f32, tag="e5f", name="e5f")
            nc.scalar.activation(out=e5f, in_=scadj, func=AF.Exp, scale=SCALE)
            den5 = stat4.tile([P, QB, T], f32, tag="den5", name="den5")
            nc.vector.tensor_reduce(out=den5, in_=e5f, axis=AX.X, op=ADD)
            rden5 = stat4.tile([P, QB, T], f32, tag="rden5", name="rden5")
            nc.vector.reciprocal(rden5, den5)
            nc.vector.tensor_tensor(
                out=e5, in0=e5f,
                in1=rden5.unsqueeze(3).to_broadcast([P, QB, T, T]), op=MULT)

            # apply + wot + out, t-outer
            for t in range(T):
                htp = tp2.tile([P, QB, C], bf16, tag="htp", name="htp%d" % t,
                               bufs=1)
                for pb in range(QB):
                    eng = nc.vector
                    acc = tp2.tile([P, C], bf16, tag="acc", name="acc%d_%d" % (t, pb),
                                   bufs=2)
                    with nc.allow_low_precision("bf16 htp"):
                        eng.tensor_scalar_mul(out=acc, in0=vp_all[:, pb, 0, :],
                                              scalar1=e5[:, pb, t, 0:1])
                        for s in range(1, T - 1):
                            eng.scalar_tensor_tensor(
                                out=acc, in0=vp_all[:, pb, s, :],
                                scalar=e5[:, pb, t, s:s + 1], in1=acc,
                                op0=MULT, op1=ADD)
                        eng.scalar_tensor_tensor(
                            out=htp[:, pb, :], in0=vp_all[:, pb, T - 1, :],
                            scalar=e5[:, pb, t, T - 1:T], in1=acc,
                            op0=MULT, op1=ADD)
                htpT = tp2.tile([P, CB, HALF], bf16, tag="htpT",
                                name="htpT%d" % t)
                for pb in range(QB):
                    nc.sync.dma_start(
                        out=htpT[:, :, pb * P:(pb + 1) * P],
                        in_=htp[:, pb, :], transpose=True)
                out_sb = tp2.tile([P, CB, HALF], bf16, tag="out_sb",
                                  name="out_sb%d" % t, bufs=1)
                tmpo2 = tp2.tile([P, CB, HALF], bf16, tag="tmpo2",
                                 name="tmpo2_%d" % t, bufs=1)
                for cb in range(0, CB, 2):
                    ps = psA.tile([P, 1024], f32, tag="ps",
                                  name="pso%d_%d" % (t, cb))
                    for dc in range(2):
                        for kc in range(CB):
                            nc.tensor.matmul(
                                ps[:, dc * 512:(dc + 1) * 512],
                                wot_sb[:, kc, (cb + dc) * P:(cb + dc + 1) * P],
                                htpT[:, kc, :],
                                start=(kc == 0), stop=(kc == CB - 1))
                    with nc.allow_low_precision("bf16 out"):
                        for dc in range(2):
                            nc.scalar.activation(
                                out=tmpo2[:, cb + dc, :],
                                in_=ps[:, dc * 512:(dc + 1) * 512],
                                func=AF.Identity, scale=1.0,
                                bias=bias_sb["bot"][:, cb + dc:cb + dc + 1])
                with nc.allow_low_precision("bf16 out"):
                    nc.gpsimd.tensor_tensor(out=out_sb, in0=tmpo2,
                                            in1=xhalfs[t], op=ADD)
                nc.sync.dma_start(
                    out=out_d[t].rearrange("(p j) hw -> p j hw", p=P),
                    in_=out_sb)

    nc.compile()
    return nc


# storage column s holds natural channel 4*(s % 128) + s // 128
_COL_PERM = np.array([4 * (s % P) + s // P for s in range(C)])


def _prepare_in_maps(inputs):
    import ml_dtypes
    x = np.asarray(inputs["x"], np.float32).reshape(B * T, C, HW)
    selbc = np.zeros((P, P), np.float32)
    for p in range(P):
        selbc[p, (p // 4) * 4:(p // 4) * 4 + 4] = 1.0
    wT8 = {}
    for nm in ["wq", "wk", "wv", "wqt", "wkt", "wvt", "wo"]:
        w = np.asarray(inputs[nm], np.float32)   # [out, in]
        wt = w.T[:, _COL_PERM] * WS              # [in, out_perm] scaled
        wT8[nm] = np.ascontiguousarray(wt).astype(ml_dtypes.float8_e4m3)
    wotT = np.ascontiguousarray(
        np.asarray(inputs["wot"], np.float32).T[:, _COL_PERM]
    ).astype(ml_dtypes.bfloat16)
    bo_eff = (np.asarray(inputs["bo"], np.float64)
              + np.asarray(inputs["wo"], np.float64)
              @ np.asarray(inputs["bv"], np.float64)).astype(np.float32)
    bot_eff = (np.asarray(inputs["bot"], np.float64)
               + np.asarray(inputs["wot"], np.float64)
               @ np.asarray(inputs["bvt"], np.float64)).astype(np.float32)
    common = {nm + "T": wT8[nm] for nm in wT8}
    common["wotT"] = wotT
    common["bq"] = np.asarray(inputs["bq"], np.float32)
    common["bk"] = np.asarray(inputs["bk"], np.float32)
    common["bo"] = bo_eff
    common["bot"] = bot_eff
    # bqt dotted against kp columns, which carry the permuted channel order
    common["bqt"] = np.asarray(inputs["bqt"], np.float32)[_COL_PERM] \
        .astype(ml_dtypes.bfloat16)
    for nm in ["gamma_s", "beta_s", "gamma_t", "beta_t"]:
        common[nm] = np.asarray(inputs[nm], np.float32)
    common["selbc"] = selbc.astype(ml_dtypes.bfloat16)

    in_maps = []
    for v in range(B):
        xv = x[v * T:(v + 1) * T]
        for h in range(2):
            if h == 0:
                xc = xv
            else:
                xc = np.concatenate([xv[..., HALF:], xv[..., :HALF]], axis=-1)
            m = dict(common)
            m["x"] = np.ascontiguousarray(xc)
            in_maps.append(m)
    return in_maps


def _run(inputs, trace=False):
    from concourse import bass_utils
    if "nc" not in _CACHE:
        _CACHE["nc"] = _build()
    nc = _CACHE["nc"]
    in_maps = _prepare_in_maps(inputs)
    if trace:
        try:
            from antenv.axon_hooks import get_axon_ntff_profile_hook  # noqa: F401
        except ModuleNotFoundError:
            trace = False
    res = bass_utils.run_bass_kernel_spmd(nc, in_maps, core_ids=list(range(8)),
                                          trace=trace)
    out = np.empty((B * T, C, HW), np.float32)
    for v in range(B):
        for h in range(2):
            o = np.asarray(res.results[2 * v + h]["out"], np.float32)
            if h == 0:
                out[v * T:(v + 1) * T, :, :HALF] = o
            else:
                out[v * T:(v + 1) * T, :, HALF:] = o
    return out.reshape(B * T, C, 32, 32), res


def kernel(**inputs) -> np.ndarray:
    out, _ = _run(inputs, trace=False)
    return out
